# revision 44
# baseline (speedup 1.0000x reference)
"""Trainium2 Bass kernel for nn_Encoder (LSTM encoder + IDM head).

Strategy:
  - Pure data parallel: batch 256 -> 8 cores x 32 rows. Tiny params replicated.
  - LSTM in transposed layout: h^T [50 units (partitions), 32 batch (free)].
  - The recurrent state h_t is written straight into the packed input tile's
    x-column region (rows 0:50, under the x rows at 64:69), so each gate's
    z = W_x x_t + b + W_h h_{t-1} is ONE K=69 matmul:
        lhsT = [rk; 0; ka+bias] [69, 50],  rhs = [h_{t-1}; 0; x_t; 1] [69, 32]
    Initial h_0 = 0 comes free from host-side zeros. 4 matmuls/step, each its
    own PSUM start/stop group.
  - Gates: ps_ifo [50, 96] (one sigmoid op over i|f|o) + ps_g [50, 32] (tanh)
    in separate PSUM banks so the sigmoid doesn't wait on the g matmul.
  - DVE pairing: g and c live adjacent in one [50, 64] tile, so
    i*g | f*c is a single tensor_tensor multiply; then one add -> c_new.
  - sigmoid/tanh/relu share one ACT table set -> single table load.
  - Dense heads: block 100 of the x region carries [h_T; 0; 0; 1], so
    preds[32,5] = matmul(lhsT=that block, rhs=W5aug[69,5]) with bias row 68.
  - IDM tail entirely on DVE (batch on partitions), with 1/(2 sqrt(ab))
    computed by bit-trick + Newton rsqrt on [32,1] to avoid a 2.7us ACT
    table-set switch.
"""

import numpy as np
from contextlib import ExitStack

import concourse.bass as bass
import concourse.tile as tile
from concourse import mybir
from concourse.bass_utils import run_bass_kernel_spmd

F32 = mybir.dt.float32
I32 = mybir.dt.int32
AF = mybir.ActivationFunctionType
OP = mybir.AluOpType

N_CORES = 8
B = 256
T = 100
U = 50            # lstm units
HP = 69           # packed tile partitions: 0:50 weights/h, 64:69 ka/x rows
BC = B // N_CORES  # batch per core = 32

# packed wts tensor column layout
C_WKA = 0                    # 200 cols: rows 0:50 = rec_kernel, 64:69 = [kernel; bias]
C_W5 = 200                   # 5 cols:   rows 0:50 = head weights, row 68 = head biases
C_SO = 205                   # 10 cols:  rows 0:32 = head post-affine S | O
C_ST = 215                   # 300 cols: rows 0:32 = vel | dv | dx
C_XT = 515                   # 101*32 cols: rows 64:69 = [x_t; 1], rows 0:50 = h scratch
WCOLS = C_XT + (T + 1) * BC  # 3747
XCOLS = (T + 1) * BC         # x payload shipped separately (rows 64:69 only)

# gate -> (psum block, weight col) ; keras weight order is i,f,g,o
IFO = [(0, 0 * U), (1, 1 * U), (2, 3 * U)]   # i, f, o in ps_ifo
GCOL = 2 * U                                  # g weights

_NC_CACHE = {}


def _drop_redundant_waits(nc):
    """Tile's wait assignment is per-proc minimal but not transitively minimal
    (a wait on engine X doesn't record what X itself had observed). walrus
    allows only one wait per instruction, so every extra wait becomes a nop.
    Compute exact happens-before vector clocks (all compute sems here are
    single-producer per-proc) and drop waits already covered transitively."""
    fn = nc.m.functions[0]
    instrs = []
    for b in fn.blocks:
        instrs.extend(b.instructions)
    n = len(instrs)

    streams = {}          # engine -> [gi...]
    pos_in_stream = {}    # gi -> (engine, pos)
    for gi, ins in enumerate(instrs):
        e = str(ins.engine)
        streams.setdefault(e, []).append(gi)
        pos_in_stream[gi] = (e, len(streams[e]) - 1)

    # sem -> producer engine(s); DMA-completed sems and multi-producer sems
    # are excluded from both crediting and dropping.
    sem_prod = {}
    sem_dma = set()
    for gi, ins in enumerate(instrs):
        si = ins.sync_info
        if not si:
            continue
        for u in (si.on_update or []):
            sem_prod.setdefault(u.ant_name, set()).add(str(ins.engine))
            if 'DMA' in type(ins).__name__ or 'Dma' in type(ins).__name__ \
                    or ins.opcode in ('DMACopy', 'DMATranspose'):
                sem_dma.add(u.ant_name)
    ok_sems = {s for s, p in sem_prod.items() if len(p) == 1 and s not in sem_dma}

    # producer events per sem: cumulative count -> producer gi
    sem_events = {}      # sem -> list of (cum_value, gi)
    for s in ok_sems:
        pe = next(iter(sem_prod[s]))
        cum = 0
        ev = []
        for gi in streams[pe]:
            si = instrs[gi].sync_info
            if si:
                for u in (si.on_update or []):
                    if u.ant_name == s:
                        cum += u.update_value
                        ev.append((cum, gi))
        sem_events[s] = ev

    def producer_gi(w):
        if w.ant_name not in ok_sems or w.wait_mode != 'sem-ge-imm' \
                or w.wait_value is None:
            return None
        for cum, gi in sem_events[w.ant_name]:
            if cum >= w.wait_value:
                return gi
        return None

    # topological sweep computing per-instruction vector clocks
    VC = [None] * n       # gi -> {engine: max stream pos}
    done = [False] * n
    ptrs = {e: 0 for e in streams}
    progress = True
    while progress:
        progress = False
        for e, sl in streams.items():
            while ptrs[e] < len(sl):
                gi = sl[ptrs[e]]
                ins = instrs[gi]
                si = ins.sync_info
                waits = list(si.on_wait) if (si and si.on_wait) else []
                deps = []
                blocked = False
                for w in waits:
                    p = producer_gi(w)
                    if p is not None:
                        if not done[p]:
                            blocked = True
                            break
                        deps.append(p)
                if blocked:
                    break
                vc = {} if ptrs[e] == 0 else dict(VC[sl[ptrs[e] - 1]])
                covered_prev = dict(vc)
                # drop waits covered by prev-stream knowledge + stronger waits
                if len(waits) > 1 and deps:
                    dep_map = {}
                    for w in waits:
                        p = producer_gi(w)
                        if p is not None:
                            dep_map[id(w)] = p
                    def cov_score(w):
                        p = dep_map.get(id(w))
                        return -1 if p is None else sum(VC[p].values()) + len(VC[p])
                    kept = []
                    covered = covered_prev
                    for w in sorted(waits, key=cov_score, reverse=True):
                        p = dep_map.get(id(w))
                        if p is None:
                            kept.append(w)
                            continue
                        ep_e, ep_pos = pos_in_stream[p]
                        if covered.get(ep_e, -1) >= ep_pos:
                            continue  # transitively covered -> drop
                        kept.append(w)
                        for k, v in VC[p].items():
                            covered[k] = max(covered.get(k, -1), v)
                        covered[ep_e] = max(covered.get(ep_e, -1), ep_pos)
                    if len(kept) < len(waits):
                        si.on_wait = kept
                    vc = covered
                else:
                    for p in deps:
                        for k, v in VC[p].items():
                            vc[k] = max(vc.get(k, -1), v)
                        ep_e, ep_pos = pos_in_stream[p]
                        vc[ep_e] = max(vc.get(ep_e, -1), ep_pos)
                vc[e] = ptrs[e]
                VC[gi] = vc
                done[gi] = True
                ptrs[e] += 1
                progress = True
    return nc


def _split_multi_waits(nc):
    """walrus codegen allows one sync-wait per engine instruction; hoist extra
    waits emitted by the Tile scheduler onto standalone EventSemaphore nops."""
    ctr = 0
    for fn in nc.m.functions:
        for b in fn.blocks:
            out = []
            changed = False
            for ins in b.instructions:
                si = ins.sync_info
                ws = list(si.on_wait) if (si is not None and si.on_wait) else []
                if len(ws) > 1:
                    for w in ws[:-1]:
                        ctr += 1
                        out.append(mybir.InstEventSemaphore(
                            name=f"I-waitsplit-{ctr}",
                            engine=ins.engine,
                            sync_info=mybir.SyncInfo(on_wait=[w], on_update=[]),
                        ))
                    si.on_wait = [ws[-1]]
                    changed = True
                out.append(ins)
            if changed:
                b.instructions = out
    return nc


def build_nc(split_waits=True):
    nc = bass.Bass(trn_type="TRN2", enable_partition_id=False)

    wts_d = nc.dram_tensor("wts", [HP, C_XT], F32, kind="ExternalInput")
    xt_d = nc.dram_tensor("xt", [19, XCOLS], F32, kind="ExternalInput")
    act_d = nc.dram_tensor("act_out", [BC, T], F32, kind="ExternalOutput")
    idm_d = nc.dram_tensor("idm_out", [BC, 5], F32, kind="ExternalOutput")

    with tile.TileContext(nc) as tc:
        with ExitStack() as ctx:
            singles = ctx.enter_context(tc.tile_pool(name="singles", bufs=1))
            temps = ctx.enter_context(tc.tile_pool(name="temps", bufs=3))
            psA = ctx.enter_context(tc.tile_pool(name="psA", bufs=3, space="PSUM"))
            psB = ctx.enter_context(tc.tile_pool(name="psB", bufs=3, space="PSUM"))
            psH = ctx.enter_context(tc.tile_pool(name="psH", bufs=1, space="PSUM"))

            # Dependency-free warmup op: pulls the ~2.7us sigmoid/tanh ACT
            # table load to t=0, overlapping it with the input DMAs.
            warm = singles.tile([1, 1], F32)
            nc.vector.memset(warm[:], 0.0)
            nc.scalar.activation(warm[:], warm[:], AF.Sigmoid)

            wts_sb = singles.tile([HP, WCOLS], F32)
            nc.sync.dma_start(wts_sb[0:HP, 0:C_XT], wts_d[:])
            nc.sync.dma_start(wts_sb[50:HP, C_XT:WCOLS], xt_d[:])
            # only h_0 (x block 0, rows 0:50) must be zero; later blocks are
            # written before read, and rows 50:64 multiply zero weight rows.
            nc.vector.memset(wts_sb[0:U, C_XT:C_XT + BC], 0.0)
            wka = wts_sb[0:HP, C_WKA:C_WKA + 4 * U]
            w5_sb = wts_sb[0:HP, C_W5:C_W5 + 5]
            so_sb = wts_sb[0:BC, C_SO:C_SO + 10]
            st_sb = wts_sb[0:BC, C_ST:C_ST + 3 * T]

            def xcol(t):
                return wts_sb[0:HP, C_XT + t * BC:C_XT + (t + 1) * BC]

            def hcol(t):
                return wts_sb[0:U, C_XT + t * BC:C_XT + t * BC + BC]

            # CC: cols 0:32 = g (written by ACT each step), cols 32:64 = c state
            CC = singles.tile([U, 2 * BC], F32)
            nc.vector.memset(CC[:], 0.0)

            vel = st_sb[:, 0:T]
            dvel = st_sb[:, T:2 * T]
            dxs = st_sb[:, 2 * T:3 * T]

            # Precompute IDM pieces that don't depend on the LSTM (fills the
            # DVE idle window at kernel start).
            veldv = singles.tile([BC, T], F32)
            nc.vector.tensor_mul(veldv[:], vel, dvel)
            rdx = singles.tile([BC, T], F32)
            nc.vector.reciprocal(rdx[:], dxs)
            vel4 = singles.tile([BC, T], F32)
            nc.vector.tensor_mul(vel4[:], vel, vel)
            nc.vector.tensor_mul(vel4[:], vel4[:], vel4[:])
            magic_sb = singles.tile([BC, 1], I32)
            nc.vector.memset(magic_sb[:], 0x5F3759DF)
            one_sb = singles.tile([BC, 1], I32)
            nc.vector.memset(one_sb[:], 1)

            # ---- LSTM over T steps ----
            for t in range(T):
                rhs = xcol(t)
                ps_g = psB.tile([U, BC], F32, tag="psg")
                nc.tensor.matmul(ps_g[:], wka[:, GCOL:GCOL + U], rhs,
                                 start=True, stop=True)
                ps_ifo = psA.tile([U, 3 * BC], F32, tag="psifo")
                for blk, wc in IFO:
                    nc.tensor.matmul(ps_ifo[:, blk * BC:(blk + 1) * BC],
                                     wka[:, wc:wc + U], rhs,
                                     start=True, stop=True)

                nc.scalar.activation(CC[:, 0:BC], ps_g[:], AF.Tanh)       # g
                sbs = temps.tile([U, 2 * BC], F32, tag="sbs")             # i|f
                nc.scalar.activation(sbs[:], ps_ifo[:, 0:2 * BC], AF.Sigmoid)

                pq = temps.tile([U, 2 * BC], F32, tag="pq")
                nc.vector.tensor_mul(pq[:], sbs[:], CC[:])                # i*g | f*c
                o_sb = temps.tile([U, BC], F32, tag="o_sb")
                nc.scalar.activation(o_sb[:], ps_ifo[:, 2 * BC:3 * BC], AF.Sigmoid)
                nc.vector.tensor_add(CC[:, BC:2 * BC], pq[:, 0:BC], pq[:, BC:2 * BC])
                tch = temps.tile([U, BC], F32, tag="tch")
                nc.scalar.activation(tch[:], CC[:, BC:2 * BC], AF.Tanh)   # tanh(c)
                nc.vector.tensor_mul(hcol(t + 1), o_sb[:], tch[:])

            # ---- dense heads: preds[32,5] in order [v, tgap, max, min, jamx]
            pp = psH.tile([BC, 5], F32, tag="pp")
            nc.tensor.matmul(pp[:], xcol(T), w5_sb, start=True, stop=True)
            t5 = temps.tile([BC, 5], F32, tag="t5")
            nc.scalar.activation(t5[:, 0:4], pp[:, 0:4], AF.Tanh)
            nc.vector.tensor_scalar(t5[:, 4:5], pp[:, 4:5], 0.0, None, op0=OP.max)
            preds = singles.tile([BC, 5], F32)
            nc.vector.tensor_mul(preds[:], t5[:], so_sb[:, 0:5])
            nc.vector.tensor_add(preds[:], preds[:], so_sb[:, 5:10])
            nc.sync.dma_start(idm_d[:], preds[:])

            dvp = preds[:, 0:1]
            tgp = preds[:, 1:2]
            mxp = preds[:, 2:3]
            mnp = preds[:, 3:4]
            jxp = preds[:, 4:5]

            # y = 0.5/sqrt(ab) via bit-trick + 2 Newton steps (last one folds 0.5)
            ab = temps.tile([BC, 1], F32, tag="s_ab")
            nc.vector.tensor_mul(ab[:], mxp, mnp)
            shi = temps.tile([BC, 1], I32, tag="s_sh")
            nc.vector.tensor_tensor(shi[:], ab[:].bitcast(I32), one_sb[:], OP.arith_shift_right)
            y0 = temps.tile([BC, 1], F32, tag="s_y0")
            nc.vector.tensor_tensor(y0[:].bitcast(I32), magic_sb[:], shi[:], OP.subtract)
            tt_ = temps.tile([BC, 1], F32, tag="s_tt")
            nc.vector.tensor_mul(tt_[:], y0[:], y0[:])
            nc.vector.tensor_mul(tt_[:], tt_[:], ab[:])
            nc.vector.tensor_scalar(tt_[:], tt_[:], -0.5, 1.5, op0=OP.mult, op1=OP.add)
            nc.vector.tensor_mul(y0[:], y0[:], tt_[:])
            nc.vector.tensor_mul(tt_[:], y0[:], y0[:])
            nc.vector.tensor_mul(tt_[:], tt_[:], ab[:])
            nc.vector.tensor_scalar(tt_[:], tt_[:], -0.25, 0.75, op0=OP.mult, op1=OP.add)
            nc.vector.tensor_mul(y0[:], y0[:], tt_[:])   # y0 = 0.5*rsqrt(ab)

            # ---- IDM over [32, 100] ----
            invd = temps.tile([BC, 1], F32, tag="s_invd")
            nc.vector.reciprocal(invd[:], dvp)
            nc.vector.tensor_mul(invd[:], invd[:], invd[:])   # invd^2
            nc.vector.tensor_mul(invd[:], invd[:], invd[:])   # invd^4
            r_ = temps.tile([BC, T], F32, tag="b_r")
            nc.vector.tensor_scalar(r_[:], vel4[:], invd[:], None, op0=OP.mult)  # r^4
            gap = temps.tile([BC, T], F32, tag="b_gap")
            nc.vector.tensor_scalar(gap[:], vel, tgp, jxp, op0=OP.mult, op1=OP.add)
            t3 = temps.tile([BC, T], F32, tag="b_t3")
            nc.vector.tensor_scalar(t3[:], veldv[:], y0[:], None, op0=OP.mult)
            nc.vector.tensor_add(gap[:], gap[:], t3[:])  # desired_gap
            nc.vector.tensor_mul(gap[:], gap[:], rdx[:])  # gap/dx
            nc.vector.tensor_mul(gap[:], gap[:], gap[:])  # (gap/dx)^2
            nc.vector.tensor_add(r_[:], r_[:], gap[:])    # r^4 + q^2
            negmx = temps.tile([BC, 1], F32, tag="s_nmx")
            nc.vector.tensor_scalar(negmx[:], mxp, -1.0, None, op0=OP.mult)
            outa = temps.tile([BC, T], F32, tag="b_out")
            nc.vector.tensor_scalar(outa[:], r_[:], negmx[:], mxp, op0=OP.mult, op1=OP.add)
            nc.sync.dma_start(act_d[:], outa[:])

    if split_waits:
        nc = _split_multi_waits(_drop_redundant_waits(nc))
    return nc


def make_in_maps(x_scaled, state, lstm_kernel, lstm_rec_kernel, lstm_bias,
                 w_desired_v, b_desired_v, w_desired_tgap, b_desired_tgap,
                 w_min_jamx, b_min_jamx, w_max_act, b_max_act, w_min_act, b_min_act):
    f32 = np.float32
    W5 = np.concatenate([w_desired_v, w_desired_tgap, w_max_act, w_min_act,
                         w_min_jamx], axis=1).astype(f32)          # [50,5]
    b5 = np.concatenate([b_desired_v, b_desired_tgap, b_max_act, b_min_act,
                         b_min_jamx]).astype(f32)                  # [5]
    # param_activation post-affines per head (order v,tgap,max,min + relu jamx)
    S5 = np.array([10.0, 1.25, 1.25, 1.75, 1.0], f32)
    O5 = np.array([25.0, 1.75, 1.75, 2.25, 0.0], f32)

    base = np.zeros((HP, C_XT), f32)
    base[0:U, C_WKA:C_WKA + 4 * U] = np.asarray(lstm_rec_kernel, f32)
    base[64:68, C_WKA:C_WKA + 4 * U] = np.asarray(lstm_kernel, f32)
    base[68, C_WKA:C_WKA + 4 * U] = np.asarray(lstm_bias, f32)
    base[0:U, C_W5:C_W5 + 5] = W5
    base[68, C_W5:C_W5 + 5] = b5
    base[0:BC, C_SO:C_SO + 5] = S5[None, :]
    base[0:BC, C_SO + 5:C_SO + 10] = O5[None, :]

    x = np.asarray(x_scaled, f32)
    st = np.asarray(state, f32)
    in_maps = []
    for c in range(N_CORES):
        xs = x[c * BC:(c + 1) * BC]                    # [32,100,4]
        ss = st[c * BC:(c + 1) * BC]
        wts = base.copy()
        wts[0:BC, C_ST:C_ST + T] = ss[:, :, 0]         # vel
        wts[0:BC, C_ST + T:C_ST + 2 * T] = ss[:, :, 2]  # dv
        wts[0:BC, C_ST + 2 * T:C_ST + 3 * T] = ss[:, :, 3]  # dx
        # rows 0:14 zero-fill partitions 50:64 (multiplied by zero weight rows,
        # but must not be NaN garbage); rows 14:18 = x features; row 18 = ones
        xt = np.zeros((19, XCOLS), f32)
        xt[14:18, 0:T * BC] = xs.transpose(2, 1, 0).reshape(4, T * BC)
        xt[18, :] = 1.0   # ones row (lstm bias; heads bias in block 100)
        in_maps.append({"wts": wts, "xt": xt})
    return in_maps


def assemble(results):
    acts = np.concatenate([r["act_out"] for r in results], axis=0)   # [256,100]
    idm = np.concatenate([r["idm_out"] for r in results], axis=0)    # [256,5]
    act_seq = np.ascontiguousarray(acts[:, :, None], dtype=np.float32)
    idm_param = np.ascontiguousarray(idm[:, [0, 1, 4, 2, 3]], dtype=np.float32)
    return act_seq, idm_param


def kernel(**inputs):
    if "nc" not in _NC_CACHE:
        _NC_CACHE["nc"] = build_nc()
    nc = _NC_CACHE["nc"]
    in_maps = make_in_maps(**inputs)
    res = run_bass_kernel_spmd(nc, in_maps, core_ids=list(range(N_CORES)))
    return assemble(res.results)
